# revision 42
# baseline (speedup 1.0000x reference)
"""Trainium2 Bass kernel for CrossImageAWGM.

Reference computation (B=16 query images, Bs=16 support images, C=512, H=W=32, P=1024):
  qc       = (w_cca_q * f_q)                      [B,C,P]
  kc_mean  = mean_b(w_cca_k * f_s)                [C,P]
  vc_mean  = mean_b(w_cca_v * f_s)                [C,P]
  Ac       = softmax_d( qc @ kc_mean^T / P )      [B,C,C]   (channel attention)
  f_cca    = Ac @ vc_mean                         [B,C,P]
  qs       = w_sca_q * f_cca
  ks_mean  = mean_b(w_sca_k * f_s)                [C,P]
  M        = qs^T @ ks_mean / (tau*C)             [B,P,P]   (never materialized)
  alpha    = softmax_p( M.mean(-1) )              [B,P]
  beta     = softmax_p( M.mean(-2) )              [B,P]
  out      = (f_q + lam*f_cca,  alpha,  broadcast(mean_b beta))

Key algebra used here:
  M.mean(-1)[p] = sum_c qs[c,p] * ksbar[c] / (tau*C)   with ksbar = ks_mean.mean(-1)
  M.mean(-2)[q] = sum_c qsbar[c] * ks_mean[c,q]/(tau*C) with qsbar = qs.mean(-1)
so the [P,P] affinity matrix reduces to two tiny matvecs.

Sharding: data-parallel over the 16 query images (2 per NeuronCore).  The
support-image mean (fs_sum) is either computed fully per-core ("replicate")
or via partial sums + an 8-core AllReduce ("allreduce").

All per-channel conv weights are folded into small host-side coefficient
vectors; matmuls run in float32r (TF32-like, full PE rate, ~1e-4 rel err).
"""

import os
import numpy as np

import concourse.bass as bass
import concourse.mybir as mybir
import concourse.tile as tile
from concourse import bacc
from concourse.bass_utils import run_bass_kernel_spmd
from concourse.masks import make_identity

F32 = mybir.dt.float32
F32R = mybir.dt.float32r
AX = mybir.AxisListType
ALU = mybir.AluOpType
ACTF = mybir.ActivationFunctionType

TAU = 0.5
N_CORES = 8

# fs_mode: "replicate" (each core reads full f_s) or "allreduce"
FS_MODE = os.environ.get("KERNEL_FS_MODE", "allreduce")


def build(BPC, Bs_local, C, P, fs_mode, n_cores):
    """Emit the SPMD kernel. Each core handles BPC query images and
    Bs_local support images. All cores identical (SPMD)."""
    CT = C // 128          # channel tiles
    PT = P // 128          # spatial tiles (partition side after transpose)
    NTILE = min(512, P)    # matmul free-dim tile (one PSUM bank of fp32)
    NT = P // NTILE

    nc = bacc.Bacc("TRN2", target_bir_lowering=False, debug=False,
                   num_devices=n_cores)

    fq_d = nc.dram_tensor("fq", [BPC, C, P], F32, kind="ExternalInput")
    fs_d = nc.dram_tensor("fs", [Bs_local, C, P], F32, kind="ExternalInput")
    # host-folded coefficient vectors (see kernel() below)
    wqb_d = nc.dram_tensor("wqb", [128, C], F32, kind="ExternalInput")
    wk_d = nc.dram_tensor("wk", [128, CT], F32, kind="ExternalInput")
    wv_d = nc.dram_tensor("wv", [128, CT], F32, kind="ExternalInput")
    hh_d = nc.dram_tensor("hh", [128, CT], F32, kind="ExternalInput")
    lam_d = nc.dram_tensor("lam", [128, 1], F32, kind="ExternalInput")

    ofq_d = nc.dram_tensor("ofq", [BPC, C, P], F32, kind="ExternalOutput")
    oal_d = nc.dram_tensor("oal", [BPC, P], F32, kind="ExternalOutput")
    obe_d = nc.dram_tensor("obe", [BPC, P], F32, kind="ExternalOutput")

    with tile.TileContext(nc) as tc:
        with tc.tile_pool(name="const", bufs=1) as cpool, \
             tc.tile_pool(name="persist", bufs=1) as perst, \
             tc.tile_pool(name="land", bufs=2) as land, \
             tc.tile_pool(name="qtp", bufs=1) as qtp, \
             tc.tile_pool(name="etp", bufs=2) as etp, \
             tc.tile_pool(name="gp", bufs=1) as gp, \
             tc.tile_pool(name="fqp", bufs=1) as fqp, \
             tc.tile_pool(name="outp", bufs=2) as outp, \
             tc.tile_pool(name="small", bufs=2) as small, \
             tc.tile_pool(name="ptr", bufs=2, space="PSUM") as ptr, \
             tc.tile_pool(name="pm1", bufs=2, space="PSUM") as pm1, \
             tc.tile_pool(name="pm2", bufs=2, space="PSUM") as pm2, \
             tc.tile_pool(name="paux", bufs=2, space="PSUM") as paux, \
             tc.tile_pool(name="dram", bufs=1, space="DRAM") as dram:

            ident = cpool.tile([128, 128], F32, tag="ident")
            make_identity(nc, ident[:])
            ones_f = cpool.tile([128, 8], F32, tag="ones_f")
            nc.vector.memset(ones_f[:], 1.0)
            ones_r = cpool.tile([128, 8], F32R, tag="ones_r")
            nc.vector.tensor_copy(ones_r[:], ones_f[:])

            wqb = cpool.tile([128, C], F32, tag="wqb")
            wk = cpool.tile([128, CT], F32, tag="wk")
            wv = cpool.tile([128, CT], F32, tag="wv")
            hh = cpool.tile([128, CT], F32, tag="hh")
            lam_t = cpool.tile([128, 1], F32, tag="lam")

            def emit_weight_dma():
                nc.sync.dma_start(out=wqb[:], in_=wqb_d[:])
                nc.sync.dma_start(out=wk[:], in_=wk_d[:])
                nc.sync.dma_start(out=wv[:], in_=wv_d[:])
                nc.sync.dma_start(out=hh[:], in_=hh_d[:])
                nc.sync.dma_start(out=lam_t[:], in_=lam_d[:])

            if fs_mode != "allreduce":
                emit_weight_dma()

            # ---- fq[0] first in the DMA queue: image-0 transposes start early ----
            fq_all, qT_all = [], []
            for b in range(BPC):
                fq = [fqp.tile([128, P], F32, tag=f"fq{b}_{ci}", name=f"fq{b}_{ci}")
                      for ci in range(CT)]
                qT = [qtp.tile([128, C], F32R, tag=f"qT{b}_{pi}", name=f"qT{b}_{pi}")
                      for pi in range(PT)]
                fq_all.append(fq)
                qT_all.append(qT)

            def emit_fq_dma(b):
                for ci in range(CT):
                    nc.sync.dma_start(out=fq_all[b][ci][:],
                                      in_=fq_d[b, ci * 128:(ci + 1) * 128, :])

            def emit_qt(b):
                for pi in range(PT):
                    pt = ptr.tile([128, C], F32, tag="ptr")
                    for ci in range(CT):
                        nc.tensor.transpose(pt[:, ci * 128:(ci + 1) * 128],
                                            fq_all[b][ci][:, pi * 128:(pi + 1) * 128],
                                            ident[:])
                    nc.vector.tensor_tensor(out=qT_all[b][pi][:], in0=pt[:],
                                            in1=wqb[:], op=ALU.mult)

            if fs_mode != "allreduce":
                # local/replicate: S is the long pole only via DMA; fq0 first
                emit_fq_dma(0)

            # ---- S = sum_b' f_s  (global over all 16 support images) ----
            S = [perst.tile([128, P], F32, tag=f"S{ci}", name=f"S{ci}") for ci in range(CT)]
            if fs_mode == "allreduce":
                cin = dram.tile([C, P], F32, tag="cin")
                cout = dram.tile([C, P], F32, tag="cout",
                                 addr_space="Shared" if n_cores > 4 else "Local")
            for ci in range(CT):
                acc = S[ci]
                lands = []
                for b in range(Bs_local):
                    lt = land.tile([128, P], F32, tag=f"land{b % 2}", name=f"land{ci}_{b}")
                    nc.sync.dma_start(out=lt[:], in_=fs_d[b, ci * 128:(ci + 1) * 128, :])
                    lands.append(lt)
                    if b == 1:
                        nc.vector.tensor_tensor(out=acc[:], in0=lands[0][:],
                                                in1=lands[1][:], op=ALU.add)
                    elif b > 1:
                        nc.vector.tensor_tensor(out=acc[:], in0=acc[:],
                                                in1=lt[:], op=ALU.add)
                if Bs_local == 1:
                    nc.vector.tensor_copy(acc[:], lands[0][:])
                if fs_mode == "allreduce":
                    # scalar-issued DMA is HWDGE on TRN2: a third queue, so the
                    # collective feed neither blocks nor rides slow SWDGE
                    nc.scalar.dma_start(out=cin[ci * 128:(ci + 1) * 128, :],
                                        in_=S[ci][:])

            if fs_mode == "allreduce":
                nc.gpsimd.collective_compute(
                    "AllReduce", ALU.add,
                    replica_groups=[list(range(n_cores))],
                    ins=[cin.opt()], outs=[cout.opt()])

            if fs_mode == "allreduce":
                # fs went first (earliest collective dispatch); weights, fq
                # streams + both images' transposes fill the latency window
                emit_weight_dma()
                emit_fq_dma(0)
                for b in range(1, BPC):
                    emit_fq_dma(b)
                for b in range(BPC):
                    emit_qt(b)
                # half-split readback: first ST transpose groups start after
                # 1MiB instead of the full 2MiB
                for h in range(2):
                    for ci in range(CT):
                        nc.sync.dma_start(
                            out=S[ci][:, h * (P // 2):(h + 1) * (P // 2)],
                            in_=cout[ci * 128:(ci + 1) * 128,
                                     h * (P // 2):(h + 1) * (P // 2)])
            else:
                emit_qt(0)
                for b in range(1, BPC):
                    emit_fq_dma(b)

            # ---- derived support-side tensors ----
            # ST = S^T  (f32r)  [P,C] as PT tiles of [128, C]
            ST = [perst.tile([128, C], F32R, tag=f"ST{pi}", name=f"ST{pi}") for pi in range(PT)]
            for pi in range(PT):
                pt = ptr.tile([128, C], F32, tag="ptr")
                for ci in range(CT):
                    nc.tensor.transpose(pt[:, ci * 128:(ci + 1) * 128],
                                        S[ci][:, pi * 128:(pi + 1) * 128], ident[:])
                if pi % 2 == 0:
                    nc.vector.tensor_copy(ST[pi][:], pt[:])
                else:
                    nc.scalar.activation(ST[pi][:], pt[:], ACTF.Copy)
            # vc = wv ⊙ S  (f32r)   (wv folds w_cca_v / Bs)
            VC = [perst.tile([128, P], F32R, tag=f"VC{ci}", name=f"VC{ci}") for ci in range(CT)]
            for ci in range(CT):
                nc.scalar.activation(VC[ci][:], S[ci][:], ACTF.Copy,
                                     scale=wv[:, ci:ci + 1])
            # S_r: f32r copy of S (beta matmul rhs), fused with rowsum for alpha coeffs
            sbar = small.tile([128, CT], F32, tag="sbar")
            SR = [perst.tile([128, P], F32R, tag=f"SR{ci}", name=f"SR{ci}") for ci in range(CT)]
            for ci in range(CT):
                nc.vector.tensor_scalar(out=SR[ci][:], in0=S[ci][:], scalar1=1.0,
                                        scalar2=None, op0=ALU.mult, op1=ALU.add,
                                        accum_out=sbar[:, ci:ci + 1])
            A_t = small.tile([128, CT], F32R, tag="A_t")
            nc.vector.tensor_tensor(out=A_t[:], in0=sbar[:], in1=hh[:], op=ALU.mult)
            # ---- per query image ----
            for b in range(BPC):
                fq = fq_all[b]
                qT = qT_all[b]
                if b > 0 and fs_mode != "allreduce":
                    emit_qt(b)

                # m1: LT[dj] = sum_pi ST[pi][:,dj]ᵀ @ qT[pi]  -> [128d, C] psum
                # then ET = exp(wk[d] * LT)  (f32r; wk folds w_cca_k/(P*Bs))
                ET = [etp.tile([128, C], F32R, tag=f"ET{dj}", name=f"ET{b}_{dj}") for dj in range(CT)]
                for dj in range(CT):
                    p1 = pm1.tile([128, C], F32, tag="pm1")
                    for pi in range(PT):
                        nc.tensor.matmul(p1[:], ST[pi][:, dj * 128:(dj + 1) * 128],
                                         qT[pi][:], start=(pi == 0), stop=(pi == PT - 1))
                    nc.scalar.activation(ET[dj][:], p1[:], ACTF.Exp,
                                         scale=wk[:, dj:dj + 1])

                # denominators: den[ci] = sum_d ET[dj][:, ci]  -> [128c, 1]
                gs = small.tile([128, CT], F32, tag="gs")
                for ci in range(CT):
                    pd = paux.tile([128, 8], F32, tag="aux", name=f"pd{b}_{ci}")
                    for dj in range(CT):
                        nc.tensor.matmul(pd[:], ET[dj][:, ci * 128:(ci + 1) * 128],
                                         ones_r[:], start=(dj == 0), stop=(dj == CT - 1))
                    rc = small.tile([128, 1], F32, tag="rc")
                    nc.vector.reciprocal(rc[:], pd[:, 0:1])
                    # gs = lam / den
                    nc.vector.tensor_tensor(out=gs[:, ci:ci + 1], in0=rc[:],
                                            in1=lam_t[:], op=ALU.mult)

                # m2: g[ci] = lam/den[c] * sum_dj ET[dj][:,ci]ᵀ @ VC[dj]   (= lam*f_cca)
                g = [gp.tile([128, P], F32R, tag=f"g{ci}", name=f"g{b}_{ci}") for ci in range(CT)]
                gacc = small.tile([128, CT * NT], F32, tag="gacc")
                for ci in range(CT):
                    for nj in range(NT):
                        p2 = pm2.tile([128, NTILE], F32, tag="pm2")
                        for dj in range(CT):
                            nc.tensor.matmul(p2[:], ET[dj][:, ci * 128:(ci + 1) * 128],
                                             VC[dj][:, nj * NTILE:(nj + 1) * NTILE],
                                             start=(dj == 0), stop=(dj == CT - 1))
                        nc.scalar.activation(
                            g[ci][:, nj * NTILE:(nj + 1) * NTILE], p2[:], ACTF.Copy,
                            scale=gs[:, ci:ci + 1],
                            accum_out=gacc[:, ci * NT + nj:ci * NT + nj + 1])
                    # stream Fq_out per channel tile as soon as g[ci] is done
                    ot = outp.tile([128, P], F32, tag="ot")
                    nc.vector.tensor_tensor(out=ot[:], in0=fq[ci][:],
                                            in1=g[ci][:].bitcast(F32), op=ALU.add)
                    nc.sync.dma_start(out=ofq_d[b, ci * 128:(ci + 1) * 128, :], in_=ot[:])

                # alpha logits = sum_c A[c] g[c,p] ; beta logits = sum_c B2[c] S[c,q]
                gsum = small.tile([128, CT], F32, tag="gsum")
                for ci in range(CT):
                    if NT > 1:
                        nc.vector.tensor_tensor(
                            out=gsum[:, ci:ci + 1],
                            in0=gacc[:, ci * NT:ci * NT + 1],
                            in1=gacc[:, ci * NT + 1:ci * NT + 2], op=ALU.add)
                    else:
                        nc.vector.tensor_copy(gsum[:, ci:ci + 1],
                                              gacc[:, ci * NT:ci * NT + 1])
                B2 = small.tile([128, CT], F32R, tag="B2")
                nc.vector.tensor_tensor(out=B2[:], in0=gsum[:], in1=hh[:], op=ALU.mult)

                for (name, lhs, rhs, od) in (("al", A_t, g, oal_d),
                                             ("be", B2, SR, obe_d)):
                    sm_e = small.tile([1, P], F32, tag=f"sm_e_{name}", bufs=1)
                    sm_s = small.tile([1, NT], F32, tag=f"sm_s_{name}")
                    for nj in range(NT):
                        pl = paux.tile([1, NTILE], F32, tag="aux", name=f"pl{b}_{nj}")
                        for ci in range(CT):
                            nc.tensor.matmul(pl[:], lhs[:, ci:ci + 1],
                                             rhs[ci][:, nj * NTILE:(nj + 1) * NTILE],
                                             start=(ci == 0), stop=(ci == CT - 1))
                        nc.scalar.activation(sm_e[:, nj * NTILE:(nj + 1) * NTILE],
                                             pl[:], ACTF.Exp,
                                             accum_out=sm_s[:, nj:nj + 1])
                    den = small.tile([1, 1], F32, tag=f"den_{name}")
                    nc.vector.tensor_reduce(den[:], sm_s[:], AX.X, ALU.add)
                    rd = small.tile([1, 1], F32, tag=f"rd_{name}")
                    nc.vector.reciprocal(rd[:], den[:])
                    nc.vector.tensor_scalar(out=sm_e[:], in0=sm_e[:], scalar1=rd[:],
                                            scalar2=None, op0=ALU.mult)
                    nc.sync.dma_start(out=od[b:b + 1, :], in_=sm_e[:])


    nc.compile()
    return nc


_BUILD_CACHE = {}


def _get_built(key, *args):
    if key not in _BUILD_CACHE:
        _BUILD_CACHE[key] = build(*args)
    return _BUILD_CACHE[key]


def kernel(f_q, f_s, w_cca_q, w_cca_k, w_cca_v, w_sca_q, w_sca_k, lam):
    f_q = np.asarray(f_q, dtype=np.float32)
    f_s = np.asarray(f_s, dtype=np.float32)
    w_cca_q = np.asarray(w_cca_q, dtype=np.float32)
    w_cca_k = np.asarray(w_cca_k, dtype=np.float32)
    w_cca_v = np.asarray(w_cca_v, dtype=np.float32)
    w_sca_q = np.asarray(w_sca_q, dtype=np.float32)
    w_sca_k = np.asarray(w_sca_k, dtype=np.float32)
    lam_f = float(np.asarray(lam))

    B, C, H, W = f_q.shape
    Bs = f_s.shape[0]
    P = H * W
    n_cores = N_CORES
    BPC = B // n_cores
    CT = C // 128
    fs_mode = FS_MODE
    Bs_local = Bs // n_cores if fs_mode == "allreduce" else Bs

    fqr = f_q.reshape(B, C, P)
    fsr = f_s.reshape(Bs, C, P)

    def to_pc(v):  # [C] -> [128, CT] partition-major
        return np.ascontiguousarray(v.reshape(CT, 128).T)

    wqb = np.ascontiguousarray(np.broadcast_to(w_cca_q[None, :], (128, C)))

    wk = to_pc(w_cca_k / (P * Bs))
    wv = to_pc(w_cca_v / Bs)
    hh = to_pc(w_sca_q * w_sca_k / (TAU * C * lam_f * P * Bs))
    lamv = np.full((128, 1), lam_f, dtype=np.float32)

    nc = _get_built((B, Bs, C, P, fs_mode), BPC, Bs_local, C, P, fs_mode, n_cores)

    in_maps = []
    for core in range(n_cores):
        fs_shard = (fsr[core * Bs_local:(core + 1) * Bs_local] if fs_mode == "allreduce"
                    else fsr)
        in_maps.append({
            "fq": np.ascontiguousarray(fqr[core * BPC:(core + 1) * BPC]),
            "fs": np.ascontiguousarray(fs_shard),
            "wqb": wqb, "wk": wk, "wv": wv, "hh": hh, "lam": lamv,
        })

    res = run_bass_kernel_spmd(nc, in_maps, core_ids=list(range(n_cores)))

    Fq_out = np.concatenate([r["ofq"] for r in res.results], axis=0)
    Fq_out = Fq_out.reshape(B, C, H, W)
    alpha = np.concatenate([r["oal"] for r in res.results], axis=0)
    betas = np.concatenate([r["obe"] for r in res.results], axis=0)
    beta_mean = np.broadcast_to(betas.mean(axis=0, keepdims=True), (Bs, P)).copy()
    return (Fq_out, alpha, beta_mean)


# revision 43
# speedup vs baseline: 1.0053x; 1.0053x over previous
"""Trainium2 Bass kernel for CrossImageAWGM.

Reference computation (B=16 query images, Bs=16 support images, C=512, H=W=32, P=1024):
  qc       = (w_cca_q * f_q)                      [B,C,P]
  kc_mean  = mean_b(w_cca_k * f_s)                [C,P]
  vc_mean  = mean_b(w_cca_v * f_s)                [C,P]
  Ac       = softmax_d( qc @ kc_mean^T / P )      [B,C,C]   (channel attention)
  f_cca    = Ac @ vc_mean                         [B,C,P]
  qs       = w_sca_q * f_cca
  ks_mean  = mean_b(w_sca_k * f_s)                [C,P]
  M        = qs^T @ ks_mean / (tau*C)             [B,P,P]   (never materialized)
  alpha    = softmax_p( M.mean(-1) )              [B,P]
  beta     = softmax_p( M.mean(-2) )              [B,P]
  out      = (f_q + lam*f_cca,  alpha,  broadcast(mean_b beta))

Key algebra used here:
  M.mean(-1)[p] = sum_c qs[c,p] * ksbar[c] / (tau*C)   with ksbar = ks_mean.mean(-1)
  M.mean(-2)[q] = sum_c qsbar[c] * ks_mean[c,q]/(tau*C) with qsbar = qs.mean(-1)
so the [P,P] affinity matrix reduces to two tiny matvecs.

Sharding: data-parallel over the 16 query images (2 per NeuronCore).  The
support-image mean (fs_sum) is either computed fully per-core ("replicate")
or via partial sums + an 8-core AllReduce ("allreduce").

All per-channel conv weights are folded into small host-side coefficient
vectors; matmuls run in float32r (TF32-like, full PE rate, ~1e-4 rel err).
"""

import os
import numpy as np

import concourse.bass as bass
import concourse.mybir as mybir
import concourse.tile as tile
from concourse import bacc
from concourse.bass_utils import run_bass_kernel_spmd
from concourse.masks import make_identity

F32 = mybir.dt.float32
F32R = mybir.dt.float32r
AX = mybir.AxisListType
ALU = mybir.AluOpType
ACTF = mybir.ActivationFunctionType

TAU = 0.5
N_CORES = 8

# fs_mode: "replicate" (each core reads full f_s) or "allreduce"
FS_MODE = os.environ.get("KERNEL_FS_MODE", "allreduce")


def build(BPC, Bs_local, C, P, fs_mode, n_cores):
    """Emit the SPMD kernel. Each core handles BPC query images and
    Bs_local support images. All cores identical (SPMD)."""
    CT = C // 128          # channel tiles
    PT = P // 128          # spatial tiles (partition side after transpose)
    NTILE = min(512, P)    # matmul free-dim tile (one PSUM bank of fp32)
    NT = P // NTILE

    nc = bacc.Bacc("TRN2", target_bir_lowering=False, debug=False,
                   num_devices=n_cores)

    fq_d = nc.dram_tensor("fq", [BPC, C, P], F32, kind="ExternalInput")
    fs_d = nc.dram_tensor("fs", [Bs_local, C, P], F32, kind="ExternalInput")
    # host-folded coefficient vectors (see kernel() below)
    wqb_d = nc.dram_tensor("wqb", [128, C], F32, kind="ExternalInput")
    wk_d = nc.dram_tensor("wk", [128, CT], F32, kind="ExternalInput")
    wv_d = nc.dram_tensor("wv", [128, CT], F32, kind="ExternalInput")
    hh_d = nc.dram_tensor("hh", [128, CT], F32, kind="ExternalInput")
    lam_d = nc.dram_tensor("lam", [128, 1], F32, kind="ExternalInput")

    ofq_d = nc.dram_tensor("ofq", [BPC, C, P], F32, kind="ExternalOutput")
    oal_d = nc.dram_tensor("oal", [BPC, P], F32, kind="ExternalOutput")
    obe_d = nc.dram_tensor("obe", [BPC, P], F32, kind="ExternalOutput")

    with tile.TileContext(nc) as tc:
        with tc.tile_pool(name="const", bufs=1) as cpool, \
             tc.tile_pool(name="persist", bufs=1) as perst, \
             tc.tile_pool(name="land", bufs=2) as land, \
             tc.tile_pool(name="qtp", bufs=1) as qtp, \
             tc.tile_pool(name="etp", bufs=2) as etp, \
             tc.tile_pool(name="gp", bufs=1) as gp, \
             tc.tile_pool(name="fqp", bufs=1) as fqp, \
             tc.tile_pool(name="outp", bufs=2) as outp, \
             tc.tile_pool(name="small", bufs=2) as small, \
             tc.tile_pool(name="ptr", bufs=2, space="PSUM") as ptr, \
             tc.tile_pool(name="pm1", bufs=2, space="PSUM") as pm1, \
             tc.tile_pool(name="pm2", bufs=2, space="PSUM") as pm2, \
             tc.tile_pool(name="paux", bufs=2, space="PSUM") as paux, \
             tc.tile_pool(name="dram", bufs=1, space="DRAM") as dram:

            ident = cpool.tile([128, 128], F32, tag="ident")
            make_identity(nc, ident[:])
            ones_f = cpool.tile([128, 8], F32, tag="ones_f")
            nc.vector.memset(ones_f[:], 1.0)
            ones_r = cpool.tile([128, 8], F32R, tag="ones_r")
            nc.vector.tensor_copy(ones_r[:], ones_f[:])

            wqb = cpool.tile([128, C], F32, tag="wqb")
            wk = cpool.tile([128, CT], F32, tag="wk")
            wv = cpool.tile([128, CT], F32, tag="wv")
            hh = cpool.tile([128, CT], F32, tag="hh")
            lam_t = cpool.tile([128, 1], F32, tag="lam")

            def emit_weight_dma():
                nc.sync.dma_start(out=wqb[:], in_=wqb_d[:])
                nc.sync.dma_start(out=wk[:], in_=wk_d[:])
                nc.sync.dma_start(out=wv[:], in_=wv_d[:])
                nc.sync.dma_start(out=hh[:], in_=hh_d[:])
                nc.sync.dma_start(out=lam_t[:], in_=lam_d[:])

            if fs_mode != "allreduce":
                emit_weight_dma()

            # ---- fq[0] first in the DMA queue: image-0 transposes start early ----
            fq_all, qT_all = [], []
            for b in range(BPC):
                fq = [fqp.tile([128, P], F32, tag=f"fq{b}_{ci}", name=f"fq{b}_{ci}")
                      for ci in range(CT)]
                qT = [qtp.tile([128, C], F32R, tag=f"qT{b}_{pi}", name=f"qT{b}_{pi}")
                      for pi in range(PT)]
                fq_all.append(fq)
                qT_all.append(qT)

            def emit_fq_dma(b):
                for ci in range(CT):
                    nc.sync.dma_start(out=fq_all[b][ci][:],
                                      in_=fq_d[b, ci * 128:(ci + 1) * 128, :])

            def emit_qt(b):
                for pi in range(PT):
                    pt = ptr.tile([128, C], F32, tag="ptr")
                    for ci in range(CT):
                        nc.tensor.transpose(pt[:, ci * 128:(ci + 1) * 128],
                                            fq_all[b][ci][:, pi * 128:(pi + 1) * 128],
                                            ident[:])
                    nc.vector.tensor_tensor(out=qT_all[b][pi][:], in0=pt[:],
                                            in1=wqb[:], op=ALU.mult)

            if fs_mode != "allreduce":
                # local/replicate: S is the long pole only via DMA; fq0 first
                emit_fq_dma(0)

            # ---- S = sum_b' f_s  (global over all 16 support images) ----
            S = [perst.tile([128, P], F32, tag=f"S{ci}", name=f"S{ci}") for ci in range(CT)]
            if fs_mode == "allreduce":
                cin = dram.tile([C, P], F32, tag="cin")
                cout = dram.tile([C, P], F32, tag="cout",
                                 addr_space="Shared" if n_cores > 4 else "Local")
            for ci in range(CT):
                acc = S[ci]
                lands = []
                for b in range(Bs_local):
                    lt = land.tile([128, P], F32, tag=f"land{b % 2}", name=f"land{ci}_{b}")
                    nc.sync.dma_start(out=lt[:], in_=fs_d[b, ci * 128:(ci + 1) * 128, :])
                    lands.append(lt)
                    if b == 1:
                        nc.vector.tensor_tensor(out=acc[:], in0=lands[0][:],
                                                in1=lands[1][:], op=ALU.add)
                    elif b > 1:
                        nc.vector.tensor_tensor(out=acc[:], in0=acc[:],
                                                in1=lt[:], op=ALU.add)
                if Bs_local == 1:
                    nc.vector.tensor_copy(acc[:], lands[0][:])
                if fs_mode == "allreduce":
                    # scalar-issued DMA is HWDGE on TRN2: a third queue, so the
                    # collective feed neither blocks nor rides slow SWDGE
                    nc.scalar.dma_start(out=cin[ci * 128:(ci + 1) * 128, :],
                                        in_=S[ci][:])

            if fs_mode == "allreduce":
                nc.gpsimd.collective_compute(
                    "AllReduce", ALU.add,
                    replica_groups=[list(range(n_cores))],
                    ins=[cin.opt()], outs=[cout.opt()])

            if fs_mode == "allreduce":
                # fs went first (earliest collective dispatch); weights, fq
                # streams + both images' transposes fill the latency window
                emit_weight_dma()
                emit_fq_dma(0)
                for b in range(1, BPC):
                    emit_fq_dma(b)
                for b in range(BPC):
                    emit_qt(b)
                # half-split readback: first ST transpose groups start after
                # 1MiB instead of the full 2MiB
                for h in range(2):
                    for ci in range(CT):
                        nc.sync.dma_start(
                            out=S[ci][:, h * (P // 2):(h + 1) * (P // 2)],
                            in_=cout[ci * 128:(ci + 1) * 128,
                                     h * (P // 2):(h + 1) * (P // 2)])
            else:
                emit_qt(0)
                for b in range(1, BPC):
                    emit_fq_dma(b)

            # ---- derived support-side tensors ----
            # ST = S^T  (f32r)  [P,C] as PT tiles of [128, C]
            ST = [perst.tile([128, C], F32R, tag=f"ST{pi}", name=f"ST{pi}") for pi in range(PT)]
            for pi in range(PT):
                pt = ptr.tile([128, C], F32, tag="ptr")
                for ci in range(CT):
                    nc.tensor.transpose(pt[:, ci * 128:(ci + 1) * 128],
                                        S[ci][:, pi * 128:(pi + 1) * 128], ident[:])
                if pi % 2 == 0:
                    nc.vector.tensor_copy(ST[pi][:], pt[:])
                else:
                    nc.scalar.activation(ST[pi][:], pt[:], ACTF.Copy)
            # vc = wv ⊙ S  (f32r)   (wv folds w_cca_v / Bs)
            VC = [perst.tile([128, P], F32R, tag=f"VC{ci}", name=f"VC{ci}") for ci in range(CT)]
            for ci in range(CT):
                nc.vector.tensor_scalar(out=VC[ci][:], in0=S[ci][:],
                                        scalar1=wv[:, ci:ci + 1], scalar2=None,
                                        op0=ALU.mult)
            # S_r: f32r copy of S (beta matmul rhs), fused with rowsum for alpha coeffs
            sbar = small.tile([128, CT], F32, tag="sbar")
            SR = [perst.tile([128, P], F32R, tag=f"SR{ci}", name=f"SR{ci}") for ci in range(CT)]
            for ci in range(CT):
                nc.vector.tensor_scalar(out=SR[ci][:], in0=S[ci][:], scalar1=1.0,
                                        scalar2=None, op0=ALU.mult, op1=ALU.add,
                                        accum_out=sbar[:, ci:ci + 1])
            A_t = small.tile([128, CT], F32R, tag="A_t")
            nc.vector.tensor_tensor(out=A_t[:], in0=sbar[:], in1=hh[:], op=ALU.mult)
            # ---- per query image ----
            for b in range(BPC):
                fq = fq_all[b]
                qT = qT_all[b]
                if b > 0 and fs_mode != "allreduce":
                    emit_qt(b)

                # m1: LT[dj] = sum_pi ST[pi][:,dj]ᵀ @ qT[pi]  -> [128d, C] psum
                # then ET = exp(wk[d] * LT)  (f32r; wk folds w_cca_k/(P*Bs))
                ET = [etp.tile([128, C], F32R, tag=f"ET{dj}", name=f"ET{b}_{dj}") for dj in range(CT)]
                for dj in range(CT):
                    p1 = pm1.tile([128, C], F32, tag="pm1")
                    for pi in range(PT):
                        nc.tensor.matmul(p1[:], ST[pi][:, dj * 128:(dj + 1) * 128],
                                         qT[pi][:], start=(pi == 0), stop=(pi == PT - 1))
                    nc.scalar.activation(ET[dj][:], p1[:], ACTF.Exp,
                                         scale=wk[:, dj:dj + 1])

                # denominators: den[ci] = sum_d ET[dj][:, ci]  -> [128c, 1]
                gs = small.tile([128, CT], F32, tag="gs")
                for ci in range(CT):
                    pd = paux.tile([128, 8], F32, tag="aux", name=f"pd{b}_{ci}")
                    for dj in range(CT):
                        nc.tensor.matmul(pd[:], ET[dj][:, ci * 128:(ci + 1) * 128],
                                         ones_r[:], start=(dj == 0), stop=(dj == CT - 1))
                    rc = small.tile([128, 1], F32, tag="rc")
                    nc.vector.reciprocal(rc[:], pd[:, 0:1])
                    # gs = lam / den
                    nc.vector.tensor_tensor(out=gs[:, ci:ci + 1], in0=rc[:],
                                            in1=lam_t[:], op=ALU.mult)

                # m2: g[ci] = lam/den[c] * sum_dj ET[dj][:,ci]ᵀ @ VC[dj]   (= lam*f_cca)
                g = [gp.tile([128, P], F32R, tag=f"g{ci}", name=f"g{b}_{ci}") for ci in range(CT)]
                gacc = small.tile([128, CT * NT], F32, tag="gacc")
                for ci in range(CT):
                    for nj in range(NT):
                        p2 = pm2.tile([128, NTILE], F32, tag="pm2")
                        for dj in range(CT):
                            nc.tensor.matmul(p2[:], ET[dj][:, ci * 128:(ci + 1) * 128],
                                             VC[dj][:, nj * NTILE:(nj + 1) * NTILE],
                                             start=(dj == 0), stop=(dj == CT - 1))
                        nc.scalar.activation(
                            g[ci][:, nj * NTILE:(nj + 1) * NTILE], p2[:], ACTF.Copy,
                            scale=gs[:, ci:ci + 1],
                            accum_out=gacc[:, ci * NT + nj:ci * NT + nj + 1])
                    # stream Fq_out per channel tile as soon as g[ci] is done
                    ot = outp.tile([128, P], F32, tag="ot")
                    nc.vector.tensor_tensor(out=ot[:], in0=fq[ci][:],
                                            in1=g[ci][:].bitcast(F32), op=ALU.add)
                    nc.sync.dma_start(out=ofq_d[b, ci * 128:(ci + 1) * 128, :], in_=ot[:])

                # alpha logits = sum_c A[c] g[c,p] ; beta logits = sum_c B2[c] S[c,q]
                gsum = small.tile([128, CT], F32, tag="gsum")
                for ci in range(CT):
                    if NT > 1:
                        nc.vector.tensor_tensor(
                            out=gsum[:, ci:ci + 1],
                            in0=gacc[:, ci * NT:ci * NT + 1],
                            in1=gacc[:, ci * NT + 1:ci * NT + 2], op=ALU.add)
                    else:
                        nc.vector.tensor_copy(gsum[:, ci:ci + 1],
                                              gacc[:, ci * NT:ci * NT + 1])
                B2 = small.tile([128, CT], F32R, tag="B2")
                nc.vector.tensor_tensor(out=B2[:], in0=gsum[:], in1=hh[:], op=ALU.mult)

                for (name, lhs, rhs, od) in (("al", A_t, g, oal_d),
                                             ("be", B2, SR, obe_d)):
                    sm_e = small.tile([1, P], F32, tag=f"sm_e_{name}", bufs=1)
                    sm_s = small.tile([1, NT], F32, tag=f"sm_s_{name}")
                    for nj in range(NT):
                        pl = paux.tile([1, NTILE], F32, tag="aux", name=f"pl{b}_{nj}")
                        for ci in range(CT):
                            nc.tensor.matmul(pl[:], lhs[:, ci:ci + 1],
                                             rhs[ci][:, nj * NTILE:(nj + 1) * NTILE],
                                             start=(ci == 0), stop=(ci == CT - 1))
                        nc.scalar.activation(sm_e[:, nj * NTILE:(nj + 1) * NTILE],
                                             pl[:], ACTF.Exp,
                                             accum_out=sm_s[:, nj:nj + 1])
                    den = small.tile([1, 1], F32, tag=f"den_{name}")
                    nc.vector.tensor_reduce(den[:], sm_s[:], AX.X, ALU.add)
                    rd = small.tile([1, 1], F32, tag=f"rd_{name}")
                    nc.vector.reciprocal(rd[:], den[:])
                    nc.vector.tensor_scalar(out=sm_e[:], in0=sm_e[:], scalar1=rd[:],
                                            scalar2=None, op0=ALU.mult)
                    nc.sync.dma_start(out=od[b:b + 1, :], in_=sm_e[:])


    nc.compile()
    return nc


_BUILD_CACHE = {}


def _get_built(key, *args):
    if key not in _BUILD_CACHE:
        _BUILD_CACHE[key] = build(*args)
    return _BUILD_CACHE[key]


def kernel(f_q, f_s, w_cca_q, w_cca_k, w_cca_v, w_sca_q, w_sca_k, lam):
    f_q = np.asarray(f_q, dtype=np.float32)
    f_s = np.asarray(f_s, dtype=np.float32)
    w_cca_q = np.asarray(w_cca_q, dtype=np.float32)
    w_cca_k = np.asarray(w_cca_k, dtype=np.float32)
    w_cca_v = np.asarray(w_cca_v, dtype=np.float32)
    w_sca_q = np.asarray(w_sca_q, dtype=np.float32)
    w_sca_k = np.asarray(w_sca_k, dtype=np.float32)
    lam_f = float(np.asarray(lam))

    B, C, H, W = f_q.shape
    Bs = f_s.shape[0]
    P = H * W
    n_cores = N_CORES
    BPC = B // n_cores
    CT = C // 128
    fs_mode = FS_MODE
    Bs_local = Bs // n_cores if fs_mode == "allreduce" else Bs

    fqr = f_q.reshape(B, C, P)
    fsr = f_s.reshape(Bs, C, P)

    def to_pc(v):  # [C] -> [128, CT] partition-major
        return np.ascontiguousarray(v.reshape(CT, 128).T)

    wqb = np.ascontiguousarray(np.broadcast_to(w_cca_q[None, :], (128, C)))

    wk = to_pc(w_cca_k / (P * Bs))
    wv = to_pc(w_cca_v / Bs)
    hh = to_pc(w_sca_q * w_sca_k / (TAU * C * lam_f * P * Bs))
    lamv = np.full((128, 1), lam_f, dtype=np.float32)

    nc = _get_built((B, Bs, C, P, fs_mode), BPC, Bs_local, C, P, fs_mode, n_cores)

    in_maps = []
    for core in range(n_cores):
        fs_shard = (fsr[core * Bs_local:(core + 1) * Bs_local] if fs_mode == "allreduce"
                    else fsr)
        in_maps.append({
            "fq": np.ascontiguousarray(fqr[core * BPC:(core + 1) * BPC]),
            "fs": np.ascontiguousarray(fs_shard),
            "wqb": wqb, "wk": wk, "wv": wv, "hh": hh, "lam": lamv,
        })

    res = run_bass_kernel_spmd(nc, in_maps, core_ids=list(range(n_cores)))

    Fq_out = np.concatenate([r["ofq"] for r in res.results], axis=0)
    Fq_out = Fq_out.reshape(B, C, H, W)
    alpha = np.concatenate([r["oal"] for r in res.results], axis=0)
    betas = np.concatenate([r["obe"] for r in res.results], axis=0)
    beta_mean = np.broadcast_to(betas.mean(axis=0, keepdims=True), (Bs, P)).copy()
    return (Fq_out, alpha, beta_mean)
